# revision 19
# baseline (speedup 1.0000x reference)
"""IntraAttention Trainium2 kernel, 8-core SPMD, collective-free.

Reference computation (N=4096 rows, d=1024):
    Q = X @ Wq.T + bq ; K = X @ Wk.T + bk ; V = X @ Wv.T + bv
    alpha = softmax(Q @ K.T / sqrt(d), axis=1)
    V_ = alpha @ V
    x = concat([V_, Q], axis=1)              # [N, 2d]
    x1 = x @ Wl.T + bl                        # [N, d]
    h = x @ Wa.T + ba                         # [N, 2d]
    out = x1 * (h[:, :d] * sigmoid(h[:, d:]))

Key algebraic restructuring (removes all collectives): every core holds
the FULL X (fp8) plus its row shard X_c, and uses
    scores = Q K.T = G @ X.T + softmax-invariant row-consts,
      with G = X_c @ Wqk + bqk, Wqk = Wq.T Wk (host-precomputed), and
    alpha V = (alpha @ X) @ Wv.T + bv      (rows of alpha sum to 1)
so K and V are never materialized or gathered.

Precision: attention path (G, scores, exp, U = exp@X, V_ = U@Wv.T, and
the V_-half of the x1/h projections) in fp8-e4m3 with DoubleRow
matmuls; the Q path stays fp16 (fp8 there blows the 2e-2 budget —
measured 5.4e-2 in emulation). End-to-end rel err ~3e-3.

Performance structure (170us vs the 189us baseline, whose trace showed
the first matmul at t=21.3us, two ~3us mid stalls, and an 8us tail):
  * All DRAM inputs are HOST-PRE-TILED into the exact SBUF tile layout,
    so every DMA descriptor is a 2-16KB contiguous per-partition run
    (the old rearranges generated 0.5-1KB strided descriptors; queue
    rate is ~45GB/s at 1-2KB rows vs 135-166GB/s at 4KB+).
  * All bulk loads dispatch up-front on the three DMA-capable engine
    queues (sync / scalar / gpsimd) in per-queue deadline order, G
    inputs first and combined into three 512KB transfers; the tiny
    bias load sits at the gpsimd queue head where it gates nothing.
  * 12 dummy matmuls on memset tiles warm the PE HAM clock gate to
    2.4GHz before the first real matmul (otherwise everything until
    t~24us runs at 1.2GHz); the PE then stays warm to the last matmul.
  * Accumulation loops finish per-output-column (last two K-steps
    m-ordered) so the PSUM->SBUF bias-adds overlap the matmul tail;
    the Q projection is m-outer so its PSUM banks don't WAR against
    the scalar engine's exp backlog from the scores loop; h-b PSUM
    banks are preallocated (per-iteration allocation cost ~400ns/m).
  * GLU tail: x1*a is folded into a fused (h+ba)*x1 DVE op during the
    h-a phase, sigmoid emits fp16, and the last output chunk is split
    into two half-R pieces on two queues (sync + scalar, scalar being
    the engine that just produced the sigmoid), shrinking the
    post-matmul serial chain from ~5us to ~2us.
"""

import numpy as np
import ml_dtypes

import concourse.bass as bass
import concourse.bacc as bacc
import concourse.tile as tile
import concourse.bass_utils as bass_utils
from concourse import mybir
from concourse.alu_op_type import AluOpType

P = 128            # partitions
D = 1024           # model dim
N = 4096           # rows
NCORES = 8
R = N // NCORES    # rows per core = 512
DC = D // P        # 128-wide d chunks = 8
C2 = D // (2 * P)  # 256-wide d chunks = 4
NT = N // P        # 128-key tiles = 32
NT2 = N // (2 * P)  # 256-key tiles = 16
TD = 2 * D
QOFF = 4 * R       # offset of the Wq half inside the flat qw tiles

F32 = mybir.dt.float32
F16 = mybir.dt.float16
F8 = mybir.dt.float8e4
DR = mybir.MatmulPerfMode.DoubleRow
E4NP = ml_dtypes.float8_e4m3fn
NDUMMY = 12

AD = AluOpType.add
MU = AluOpType.mult


def build_nc():
    nc = bacc.Bacc(
        "TRN2",
        target_bir_lowering=False,
        debug=False,
        num_devices=NCORES,
    )

    # ---- per-core I/O (all host-pre-tiled; see make_in_maps) ----
    wqk_p = nc.dram_tensor("wqk_p", [2 * P, 4 * D], F8, kind="ExternalInput")
    xc8_p = nc.dram_tensor("xc8_p", [2 * P, 4 * R], F8, kind="ExternalInput")
    xk_p = nc.dram_tensor("xk_p", [8 * P, 8 * 4 * P], F8, kind="ExternalInput")
    xv_p = nc.dram_tensor("xv_p", [4 * P, 8 * D], F8, kind="ExternalInput")
    qw_p = nc.dram_tensor("qw_p", [2 * P, QOFF + 4 * D], F16, kind="ExternalInput")
    w8_p = nc.dram_tensor("w8_p", [4 * P, 8 * D], F8, kind="ExternalInput")
    w16_p = nc.dram_tensor("w16_p", [3 * P, 8 * D], F16, kind="ExternalInput")
    bias_p = nc.dram_tensor("bias_p", [P, 48], F32, kind="ExternalInput")
    out = nc.dram_tensor("out", [D, R], F16, kind="ExternalOutput")

    with tile.TileContext(nc) as tc:
        with (
            tc.tile_pool(name="cpool", bufs=1) as cpool,
            tc.tile_pool(name="pspool", bufs=8, space="PSUM") as pspool,
        ):
            bias_t = cpool.tile([P, 48], F32, name="bias_t")
            ones8 = cpool.tile([P, 2, P], F8, name="ones8")
            ones_row = cpool.tile([1, P], F32, name="ones_row")
            warm8 = cpool.tile([P, 2, R], F8, name="warm8")
            junk = cpool.tile([1, 1], F32, name="junk")
            recip_t = cpool.tile([1, R], F32, name="recip_t")

            with (
                tc.tile_pool(name="qpool", bufs=1) as qpool,
                tc.tile_pool(name="e8pool", bufs=1) as e8pool,
                tc.tile_pool(name="fpool", bufs=1) as fpool,
            ):
                qt_t = [qpool.tile([P, R], F16, name=f"qt{m}") for m in range(DC)]
                g8 = [qpool.tile([P, 2, R], F8, name=f"g8_{c}") for c in range(C2)]
                e8 = [e8pool.tile([P, 2, R], F8, name=f"e8_{i}") for i in range(NT2)]
                u8 = [fpool.tile([P, 2, R], F8, name=f"u8_{c}") for c in range(C2)]
                v8 = [fpool.tile([P, 2, R], F8, name=f"v8_{c}") for c in range(C2)]
                x1_t = [fpool.tile([P, R], F16, name=f"x1_{m}") for m in range(DC)]
                bc_t = fpool.tile([P, R], F32, name="bc_t")
                xv_a = [fpool.tile([P, 2 * C2, D], F8, name=f"xv{gr}")
                        for gr in range(NT2 // 4)]

                with (
                    tc.tile_pool(name="xpool", bufs=1) as xpool,
                    tc.tile_pool(name="skpool", bufs=1) as skpool,
                ):
                    wqk_h = [xpool.tile([P, 4, D], F8, name=f"wqk_h{h}")
                             for h in range(2)]
                    xc8_h = [xpool.tile([P, 4, R], F8, name=f"xc8_h{h}")
                             for h in range(2)]
                    qw_h = [xpool.tile([P, QOFF + 4 * D], F16, name=f"qw_h{h}")
                            for h in range(2)]
                    xk_t = [skpool.tile([P, 2 * C2, 4 * P], F8, name=f"xk{g}")
                            for g in range(NT // 4)]

                    def ld_wqk(eng, h):
                        eng.dma_start(
                            wqk_h[h],
                            wqk_p[h * P:(h + 1) * P, :]
                            .rearrange("p (c e) -> p c e", c=4))

                    def ld_xc8(eng, h):
                        eng.dma_start(
                            xc8_h[h],
                            xc8_p[h * P:(h + 1) * P, :]
                            .rearrange("p (c r) -> p c r", c=4))

                    def ld_xk(eng, g):
                        eng.dma_start(
                            xk_t[g],
                            xk_p[g * P:(g + 1) * P, :]
                            .rearrange("p (c k) -> p c k", c=2 * C2))

                    def ld_xv(eng, gr):
                        eng.dma_start(
                            xv_a[gr],
                            xv_p[gr * P:(gr + 1) * P, :]
                            .rearrange("p (t e) -> p t e", t=2 * C2))

                    def ld_qw(eng, h):
                        eng.dma_start(qw_h[h], qw_p[h * P:(h + 1) * P, :])

                    # --- head dispatches: only sync/scalar/gpsimd can issue
                    # DMAs; per-queue deadline order ---
                    nc.vector.memset(ones8, 1.0)
                    nc.vector.memset(warm8, 1.0)
                    nc.vector.memset(ones_row, 1.0)

                    ld_wqk(nc.sync, 0)
                    ld_wqk(nc.sync, 1)
                    ld_xk(nc.sync, 0)
                    ld_xk(nc.sync, 3)
                    ld_xk(nc.sync, 7)
                    ld_qw(nc.sync, 0)
                    ld_xv(nc.sync, 0)
                    ld_xv(nc.sync, 3)

                    ld_xc8(nc.scalar, 0)
                    ld_xk(nc.scalar, 1)
                    ld_xk(nc.scalar, 5)
                    ld_qw(nc.scalar, 1)
                    ld_xv(nc.scalar, 1)

                    nc.gpsimd.dma_start(bias_t, bias_p[:, :])
                    ld_xc8(nc.gpsimd, 1)
                    ld_xk(nc.gpsimd, 2)
                    ld_xk(nc.gpsimd, 4)
                    ld_xk(nc.gpsimd, 6)
                    ld_xv(nc.gpsimd, 2)

                    # --- PE warm-up: dummy matmuls on memset tiles keep the
                    # HAM activity monitor busy so real work runs at 2.4GHz ---
                    warm_ps = pspool.tile([P, R], F32, name="warm_ps", tag="ps")
                    for _ in range(NDUMMY):
                        nc.tensor.matmul(
                            warm_ps, ones8, warm8, start=True, stop=True,
                            perf_mode=DR, skip_group_check=True)
                    nc.vector.tensor_copy(junk, warm_ps[0:1, 0:1])

                    # ============ G.T = (X_c @ Wqk + bqk).T (fp8 DR) ============
                    # c-outer for chunks 0/1 (start as data lands), then the
                    # last two chunks m-ordered so each g8 bias-add overlaps
                    # the remaining matmul tail.
                    g_ps = [pspool.tile([P, R], F32, name=f"gps{m}", tag="ps")
                            for m in range(DC)]

                    def g_stat(c, m):
                        lo = 2 * (c % 2)
                        return wqk_h[c // 2][:, lo:lo + 2, m * P:(m + 1) * P]

                    def g_mov(c):
                        lo = 2 * (c % 2)
                        return xc8_h[c // 2][:, lo:lo + 2, :]

                    for c in (0, 1):
                        for m in range(DC):
                            nc.tensor.matmul(
                                g_ps[m], g_stat(c, m), g_mov(c),
                                start=(c == 0), stop=False, perf_mode=DR)
                    for m in range(DC):
                        for c in (2, 3):
                            nc.tensor.matmul(
                                g_ps[m], g_stat(c, m), g_mov(c),
                                start=False, stop=(c == 3), perf_mode=DR)
                        if m % 2 == 0:
                            nc.vector.tensor_scalar_add(
                                g8[m // 2][:, m % 2, :], g_ps[m],
                                bias_t[:, m:m + 1])
                        else:
                            nc.scalar.add(
                                g8[m // 2][:, m % 2, :], g_ps[m],
                                bias_t[:, m:m + 1])

                    # ============ scores.T -> exp (fp8 DR) + sums ============
                    sums_ps = pspool.tile([P, R], F32, name="sums_ps", tag="ps")

                    def sums_mm(i):
                        nc.tensor.matmul(
                            sums_ps, ones8, e8[i],
                            start=(i == 0), stop=(i == NT2 - 1),
                            perf_mode=DR, skip_group_check=True)

                    scs = [pspool.tile([P, R], F32, name=f"sc{i}", tag="ps")
                           for i in range(7)]
                    for t in range(NT):
                        g, u = divmod(t, 4)
                        sc_ps = scs[t % 7]
                        for c in range(C2):
                            nc.tensor.matmul(
                                sc_ps,
                                xk_t[g][:, 2 * c:2 * c + 2, u * P:(u + 1) * P],
                                g8[c],
                                start=(c == 0), stop=(c == C2 - 1), perf_mode=DR)
                        nc.scalar.activation(
                            e8[t // 2][:, t % 2, :], sc_ps,
                            mybir.ActivationFunctionType.Exp,
                            bias=0.0, scale=1.0 / 32.0)
                        if t % 2 == 1 and t >= 3:
                            sums_mm((t - 3) // 2)   # one behind: that pair is done
                    sums_mm(NT2 - 1)
                    nc.vector.reciprocal(recip_t, sums_ps[0:1, :])

                    # ============ Q = (X_c @ Wq.T + bq).T (fp16) ============
                    # Runs while the scalar engine drains the exp tail and the
                    # DVE computes the reciprocal. k-outer; the last two
                    # k-steps go m-ordered so the bias-adds overlap.
                    q_ps = [pspool.tile([P, R], F32, name=f"qps{m}", tag="ps")
                            for m in range(DC)]

                    def q_stat(k, m):
                        h, kk = divmod(k, DC // 2)
                        lo = QOFF + kk * D + m * P
                        return qw_h[h][:, lo:lo + P]

                    def q_mov(k):
                        h, kk = divmod(k, DC // 2)
                        return qw_h[h][:, kk * R:(kk + 1) * R]

                    for m in range(DC):
                        for k in range(DC):
                            nc.tensor.matmul(
                                q_ps[m], q_stat(k, m), q_mov(k),
                                start=(k == 0), stop=(k == DC - 1))
                        if m % 2 == 0:
                            nc.vector.tensor_scalar_add(
                                qt_t[m], q_ps[m], bias_t[:, 8 + m:9 + m])
                        else:
                            nc.scalar.add(qt_t[m], q_ps[m], bias_t[:, 8 + m:9 + m])
                        if m == 0:
                            # broadcast 1/sums to all partitions here: recip is
                            # ready (no PE wait) and the bank+copy are long done
                            # before the U loop's PSUM allocations
                            bc_ps = pspool.tile([P, R], F32, name="bc_ps",
                                                tag="ps")
                            nc.tensor.matmul(bc_ps, ones_row, recip_t,
                                             start=True, stop=True)
                            nc.vector.tensor_copy(bc_t, bc_ps)

                with (
                    tc.tile_pool(name="lwpool", bufs=4) as lwpool,
                    tc.tile_pool(name="fwpool", bufs=3) as fwpool,
                ):
                    # late-phase weights (pre-tiled; DMAs start as soon as the
                    # early pools' SBUF frees up)
                    wv_a = lwpool.tile([P, 2 * C2, D], F8, name="wv_a", tag="lw")
                    wlv_a = lwpool.tile([P, 2 * C2, D], F8, name="wlv_a", tag="lw")
                    wav_a = lwpool.tile([P, 2 * C2, D], F8, name="wav_a", tag="lw")
                    wag_a = lwpool.tile([P, 2 * C2, D], F8, name="wag_a", tag="lw")
                    wl_a = fwpool.tile([P, DC, D], F16, name="wl_a", tag="fw")
                    wa_a = fwpool.tile([P, DC, D], F16, name="wa_a", tag="fw")
                    wg_a = fwpool.tile([P, DC, D], F16, name="wg_a", tag="fw")

                    def ld_w8(eng, tile_, b):
                        eng.dma_start(
                            tile_,
                            w8_p[b * P:(b + 1) * P, :]
                            .rearrange("p (c e) -> p c e", c=2 * C2))

                    def ld_w16(eng, tile_, b):
                        eng.dma_start(
                            tile_,
                            w16_p[b * P:(b + 1) * P, :]
                            .rearrange("p (k e) -> p k e", k=DC))

                    ld_w8(nc.sync, wv_a, 0)
                    ld_w16(nc.sync, wl_a, 0)
                    ld_w8(nc.scalar, wlv_a, 1)
                    ld_w8(nc.scalar, wag_a, 3)
                    ld_w8(nc.gpsimd, wav_a, 2)
                    ld_w16(nc.gpsimd, wa_a, 1)
                    ld_w16(nc.gpsimd, wg_a, 2)

                    # ============ U.T = (exp @ X).T (fp8 DR), normalize ========
                    vt_ps = [pspool.tile([P, R], F32, name=f"vtps{m}", tag="ps")
                             for m in range(DC)]
                    for t in range(NT2 - 1):
                        gr, u = divmod(t, 4)
                        for m in range(DC):
                            nc.tensor.matmul(
                                vt_ps[m],
                                xv_a[gr][:, 2 * u:2 * u + 2, m * P:(m + 1) * P],
                                e8[t],
                                start=(t == 0), stop=False,
                                perf_mode=DR, skip_group_check=True)
                    for m in range(DC):
                        nc.tensor.matmul(
                            vt_ps[m],
                            xv_a[3][:, 6:8, m * P:(m + 1) * P],
                            e8[NT2 - 1],
                            start=False, stop=True,
                            perf_mode=DR, skip_group_check=True)
                        nc.vector.tensor_mul(u8[m // 2][:, m % 2, :], vt_ps[m], bc_t)

                    # ============ V_.T = (U @ Wv.T + bv).T (fp8 DR) ============
                    v_ps = [pspool.tile([P, R], F32, name=f"vps{m}", tag="ps")
                            for m in range(DC)]
                    for c in (0, 1):
                        for m in range(DC):
                            nc.tensor.matmul(
                                v_ps[m], wv_a[:, 2 * c:2 * c + 2, m * P:(m + 1) * P],
                                u8[c],
                                start=(c == 0), stop=False, perf_mode=DR)
                    for m in range(DC):
                        for c in (2, 3):
                            nc.tensor.matmul(
                                v_ps[m], wv_a[:, 2 * c:2 * c + 2, m * P:(m + 1) * P],
                                u8[c],
                                start=False, stop=(c == 3), perf_mode=DR)
                        if m % 2 == 0:
                            nc.vector.tensor_scalar_add(
                                v8[m // 2][:, m % 2, :], v_ps[m],
                                bias_t[:, 16 + m:17 + m])
                        else:
                            nc.scalar.add(
                                v8[m // 2][:, m % 2, :], v_ps[m],
                                bias_t[:, 16 + m:17 + m])

                    # ========= x1 = [V_, Q] @ Wl.T + bl  (DR + fp16) =========
                    x1_ps = [pspool.tile([P, R], F32, name=f"x1ps{m}", tag="ps")
                             for m in range(DC)]
                    for c in range(C2):
                        for m in range(DC):
                            nc.tensor.matmul(
                                x1_ps[m],
                                wlv_a[:, 2 * c:2 * c + 2, m * P:(m + 1) * P], v8[c],
                                start=(c == 0), stop=False, perf_mode=DR)
                    for k in range(DC - 2):
                        for m in range(DC):
                            nc.tensor.matmul(
                                x1_ps[m], wl_a[:, k, m * P:(m + 1) * P], qt_t[k],
                                start=False, stop=False)
                    for m in range(DC):
                        for k in (DC - 2, DC - 1):
                            nc.tensor.matmul(
                                x1_ps[m], wl_a[:, k, m * P:(m + 1) * P], qt_t[k],
                                start=False, stop=(k == DC - 1))
                        if m % 2 == 0:
                            nc.vector.tensor_scalar_add(
                                x1_t[m], x1_ps[m], bias_t[:, 24 + m:25 + m])
                        else:
                            nc.scalar.add(x1_t[m], x1_ps[m], bias_t[:, 24 + m:25 + m])

                    # ========= h a-part; fold x1 <- x1 * (h_a + ba) =========
                    h_ps = [pspool.tile([P, R], F32, name=f"hps{m}", tag="ps")
                            for m in range(DC)]
                    for c in range(C2):
                        for m in range(DC):
                            nc.tensor.matmul(
                                h_ps[m],
                                wav_a[:, 2 * c:2 * c + 2, m * P:(m + 1) * P], v8[c],
                                start=(c == 0), stop=False, perf_mode=DR)
                    for k in range(DC - 2):
                        for m in range(DC):
                            nc.tensor.matmul(
                                h_ps[m], wa_a[:, k, m * P:(m + 1) * P], qt_t[k],
                                start=False, stop=False)
                    for m in range(DC):
                        for k in (DC - 2, DC - 1):
                            nc.tensor.matmul(
                                h_ps[m], wa_a[:, k, m * P:(m + 1) * P], qt_t[k],
                                start=False, stop=(k == DC - 1))
                        nc.vector.scalar_tensor_tensor(
                            x1_t[m], h_ps[m], bias_t[:, 32 + m:33 + m], x1_t[m],
                            op0=AD, op1=MU)

                    # ===== h b-part m-outer, sigmoid from PSUM, GLU, out =====
                    out_eng = [nc.sync, nc.gpsimd, nc.scalar, nc.gpsimd,
                               nc.scalar, nc.gpsimd, nc.scalar, None]
                    # last chunk halves: sync, then scalar (same engine as the
                    # sigmoid, so the dispatch follows it with no cross-engine
                    # hop)
                    hg1s = [pspool.tile([P, R], F32, name=f"hg1_{m}", tag="ps")
                            for m in range(DC)]
                    with tc.tile_pool(name="gpool", bufs=4) as gpool:
                        for m in range(DC):
                            hg1 = hg1s[m]
                            for c in range(C2):
                                nc.tensor.matmul(
                                    hg1,
                                    wag_a[:, 2 * c:2 * c + 2, m * P:(m + 1) * P],
                                    v8[c],
                                    start=(c == 0), stop=False, perf_mode=DR)
                            for k in range(DC):
                                nc.tensor.matmul(
                                    hg1, wg_a[:, k, m * P:(m + 1) * P], qt_t[k],
                                    start=False, stop=(k == DC - 1))
                            if m < DC - 1:
                                sig = gpool.tile([P, R], F16, name="sig", tag="g")
                                nc.scalar.activation(
                                    sig, hg1,
                                    mybir.ActivationFunctionType.Sigmoid,
                                    bias=bias_t[:, 40 + m:41 + m], scale=1.0)
                                og = gpool.tile([P, R], F16, name="og", tag="g")
                                nc.vector.tensor_mul(og, x1_t[m], sig)
                                out_eng[m].dma_start(
                                    out[m * P:(m + 1) * P, :], og)
                            else:
                                # last chunk: asymmetric 384/128 split so the
                                # final exposed piece is minimal
                                for hh, (lo, hi) in enumerate([(0, 384),
                                                               (384, R)]):
                                    sig = gpool.tile([P, hi - lo], F16,
                                                     name="sigh", tag="g")
                                    nc.scalar.activation(
                                        sig, hg1[:, lo:hi],
                                        mybir.ActivationFunctionType.Sigmoid,
                                        bias=bias_t[:, 40 + m:41 + m], scale=1.0)
                                    og = gpool.tile([P, hi - lo], F16,
                                                    name="ogh", tag="g")
                                    nc.vector.tensor_mul(
                                        og, x1_t[m][:, lo:hi], sig)
                                    eng = nc.sync if hh == 0 else nc.scalar
                                    eng.dma_start(
                                        out[m * P:(m + 1) * P, lo:hi], og)

    nc.compile()
    return nc


_NC = None


def _get_nc():
    global _NC
    if _NC is None:
        _NC = build_nc()
    return _NC


def make_in_maps(input_features, Wq, bq, Wk, bk, Wv, bv, Wl, bl, Wa, ba):
    f = np.ascontiguousarray
    x = np.asarray(input_features, dtype=np.float32)            # [N, D]
    x8 = x.astype(E4NP)                                          # [N, D]
    xt8 = x8.T                                                   # [D, N] view

    wq = np.asarray(Wq, np.float32)
    wk = np.asarray(Wk, np.float32)
    wqk8 = (wq.T @ wk).astype(E4NP)                              # [D, D]
    wqk_p = f(wqk8.reshape(2, 2, 2, P, D).transpose(0, 3, 1, 2, 4)
              .reshape(2 * P, 4 * D))
    xk_p = f(np.asarray(xt8).reshape(2 * C2, P, 8, 4 * P)
             .transpose(2, 1, 0, 3).reshape(8 * P, 8 * 4 * P))
    xv_p = f(x8.reshape(4, 2 * C2, P, D).transpose(0, 2, 1, 3)
             .reshape(4 * P, 8 * D))

    wvt = np.asarray(Wv, np.float32).T                           # [D, D]
    wlt = np.asarray(Wl, np.float32).T                           # [2D, D]
    wat = np.asarray(Wa, np.float32).T                           # [2D, 2D]

    def t8(w):
        return w.astype(E4NP).reshape(DC, P, D).transpose(1, 0, 2).reshape(P, DC * D)

    def t16(w):
        return (w.astype(np.float16).reshape(DC, P, D)
                .transpose(1, 0, 2).reshape(P, DC * D))

    w8_p = f(np.concatenate(
        [t8(wvt), t8(wlt[:D]), t8(wat[:D, :D]), t8(wat[:D, D:])], axis=0))
    w16_p = f(np.concatenate(
        [t16(wlt[D:]), t16(wat[D:, :D]), t16(wat[D:, D:])], axis=0))

    bqk = np.asarray(bq, np.float32) @ wk
    bias_arr = np.zeros((P, 48), np.float32)
    bias_arr[:, 0:8] = bqk.reshape(DC, P).T
    bias_arr[:, 8:16] = np.asarray(bq, np.float32).reshape(DC, P).T
    bias_arr[:, 16:24] = np.asarray(bv, np.float32).reshape(DC, P).T
    bias_arr[:, 24:32] = np.asarray(bl, np.float32).reshape(DC, P).T
    bias_arr[:, 32:48] = np.asarray(ba, np.float32).reshape(2 * DC, P).T
    bias_arr = f(bias_arr)

    wq16t = wq.T.astype(np.float16)                              # [D, D]
    wq_part = (wq16t.reshape(2, DC // 2, P, D).transpose(0, 2, 1, 3)
               .reshape(2 * P, (DC // 2) * D))                   # [256, 4D]
    xt16 = x.T.astype(np.float16)                                # [D, N]

    in_maps = []
    for c in range(NCORES):
        xc8 = np.asarray(xt8[:, c * R:(c + 1) * R])              # [D, R]
        xc8_pc = f(xc8.reshape(2, 2, 2, P, R).transpose(0, 3, 1, 2, 4)
                   .reshape(2 * P, 4 * R))
        xtc16 = xt16[:, c * R:(c + 1) * R]                       # [D, R]
        xt_part = (xtc16.reshape(2, DC // 2, P, R).transpose(0, 2, 1, 3)
                   .reshape(2 * P, (DC // 2) * R))
        qw_pc = f(np.concatenate([xt_part, wq_part], axis=1))    # [256, 4R+4D]
        in_maps.append({
            "wqk_p": wqk_p, "xc8_p": xc8_pc, "xk_p": xk_p, "xv_p": xv_p,
            "qw_p": qw_pc, "w8_p": w8_p, "w16_p": w16_p, "bias_p": bias_arr,
        })
    return in_maps


def run(in_maps, trace=False):
    nc = _get_nc()
    return bass_utils.run_bass_kernel_spmd(
        nc, in_maps, core_ids=list(range(NCORES)), trace=trace)


def kernel(input_features, Wq, bq, Wk, bk, Wv, bv, Wl, bl, Wa, ba):
    in_maps = make_in_maps(input_features, Wq, bq, Wk, bk, Wv, bv, Wl, bl, Wa, ba)
    res = run(in_maps)
    out = np.empty((N, D), dtype=np.float32)
    for c in range(NCORES):
        out[c * R:(c + 1) * R, :] = res.results[c]["out"].T.astype(np.float32)
    return out


# revision 20
# speedup vs baseline: 1.0083x; 1.0083x over previous
"""IntraAttention Trainium2 kernel, 8-core SPMD, collective-free.

Reference computation (N=4096 rows, d=1024):
    Q = X @ Wq.T + bq ; K = X @ Wk.T + bk ; V = X @ Wv.T + bv
    alpha = softmax(Q @ K.T / sqrt(d), axis=1)
    V_ = alpha @ V
    x = concat([V_, Q], axis=1)              # [N, 2d]
    x1 = x @ Wl.T + bl                        # [N, d]
    h = x @ Wa.T + ba                         # [N, 2d]
    out = x1 * (h[:, :d] * sigmoid(h[:, d:]))

Key algebraic restructuring (removes all collectives): every core holds
the FULL X (fp8) plus its row shard X_c, and uses
    scores = Q K.T = G @ X.T + softmax-invariant row-consts,
      with G = X_c @ Wqk + bqk, Wqk = Wq.T Wk (host-precomputed), and
    alpha V = (alpha @ X) @ Wv.T + bv      (rows of alpha sum to 1)
so K and V are never materialized or gathered.

Precision: attention path (G, scores, exp, U = exp@X, V_ = U@Wv.T, and
the V_-half of the x1/h projections) in fp8-e4m3 with DoubleRow
matmuls; the Q path stays fp16 (fp8 there blows the 2e-2 budget —
measured 5.4e-2 in emulation). End-to-end rel err ~3e-3.

Performance structure (170us vs the 189us baseline, whose trace showed
the first matmul at t=21.3us, two ~3us mid stalls, and an 8us tail):
  * All DRAM inputs are HOST-PRE-TILED into the exact SBUF tile layout,
    so every DMA descriptor is a 2-16KB contiguous per-partition run
    (the old rearranges generated 0.5-1KB strided descriptors; queue
    rate is ~45GB/s at 1-2KB rows vs 135-166GB/s at 4KB+).
  * All bulk loads dispatch up-front on the three DMA-capable engine
    queues (sync / scalar / gpsimd) in per-queue deadline order, G
    inputs first and combined into three 512KB transfers; the tiny
    bias load sits at the gpsimd queue head where it gates nothing.
  * 12 dummy matmuls on memset tiles warm the PE HAM clock gate to
    2.4GHz before the first real matmul (otherwise everything until
    t~24us runs at 1.2GHz); the PE then stays warm to the last matmul.
  * Accumulation loops finish per-output-column (last two K-steps
    m-ordered) so the PSUM->SBUF bias-adds overlap the matmul tail;
    the Q projection is m-outer so its PSUM banks don't WAR against
    the scalar engine's exp backlog from the scores loop; h-b PSUM
    banks are preallocated (per-iteration allocation cost ~400ns/m).
  * GLU tail: x1*a is folded into a fused (h+ba)*x1 DVE op during the
    h-a phase, sigmoid emits fp16, and the last output chunk is split
    into two half-R pieces on two queues (sync + scalar, scalar being
    the engine that just produced the sigmoid), shrinking the
    post-matmul serial chain from ~5us to ~2us.
"""

import numpy as np
import ml_dtypes

import concourse.bass as bass
import concourse.bacc as bacc
import concourse.tile as tile
import concourse.bass_utils as bass_utils
from concourse import mybir
from concourse.alu_op_type import AluOpType

P = 128            # partitions
D = 1024           # model dim
N = 4096           # rows
NCORES = 8
R = N // NCORES    # rows per core = 512
DC = D // P        # 128-wide d chunks = 8
C2 = D // (2 * P)  # 256-wide d chunks = 4
NT = N // P        # 128-key tiles = 32
NT2 = N // (2 * P)  # 256-key tiles = 16
TD = 2 * D
QOFF = 4 * R       # offset of the Wq half inside the flat qw tiles

F32 = mybir.dt.float32
F16 = mybir.dt.float16
F8 = mybir.dt.float8e4
DR = mybir.MatmulPerfMode.DoubleRow
E4NP = ml_dtypes.float8_e4m3fn
NDUMMY = 12

AD = AluOpType.add
MU = AluOpType.mult


def build_nc():
    nc = bacc.Bacc(
        "TRN2",
        target_bir_lowering=False,
        debug=False,
        num_devices=NCORES,
    )

    # ---- per-core I/O (all host-pre-tiled; see make_in_maps) ----
    wqk_p = nc.dram_tensor("wqk_p", [2 * P, 4 * D], F8, kind="ExternalInput")
    xc8_p = nc.dram_tensor("xc8_p", [2 * P, 4 * R], F8, kind="ExternalInput")
    xk_p = nc.dram_tensor("xk_p", [8 * P, 8 * 4 * P], F8, kind="ExternalInput")
    xv_p = nc.dram_tensor("xv_p", [4 * P, 8 * D], F8, kind="ExternalInput")
    qw_p = nc.dram_tensor("qw_p", [2 * P, QOFF + 4 * D], F16, kind="ExternalInput")
    w8_p = nc.dram_tensor("w8_p", [4 * P, 8 * D], F8, kind="ExternalInput")
    w16_p = nc.dram_tensor("w16_p", [3 * P, 8 * D], F16, kind="ExternalInput")
    bias_p = nc.dram_tensor("bias_p", [P, 48], F32, kind="ExternalInput")
    out = nc.dram_tensor("out", [D, R], F16, kind="ExternalOutput")

    with tile.TileContext(nc) as tc:
        with (
            tc.tile_pool(name="cpool", bufs=1) as cpool,
            tc.tile_pool(name="pspool", bufs=8, space="PSUM") as pspool,
        ):
            bias_t = cpool.tile([P, 48], F32, name="bias_t")
            ones8 = cpool.tile([P, 2, P], F8, name="ones8")
            ones_row = cpool.tile([1, P], F32, name="ones_row")
            warm8 = cpool.tile([P, 2, R], F8, name="warm8")
            junk = cpool.tile([1, 1], F32, name="junk")
            recip_t = cpool.tile([1, R], F32, name="recip_t")

            with (
                tc.tile_pool(name="qpool", bufs=1) as qpool,
                tc.tile_pool(name="e8pool", bufs=1) as e8pool,
                tc.tile_pool(name="fpool", bufs=1) as fpool,
            ):
                qt_t = [qpool.tile([P, R], F16, name=f"qt{m}") for m in range(DC)]
                g8 = [qpool.tile([P, 2, R], F8, name=f"g8_{c}") for c in range(C2)]
                e8 = [e8pool.tile([P, 2, R], F8, name=f"e8_{i}") for i in range(NT2)]
                u8 = [fpool.tile([P, 2, R], F8, name=f"u8_{c}") for c in range(C2)]
                v8 = [fpool.tile([P, 2, R], F8, name=f"v8_{c}") for c in range(C2)]
                x1_t = [fpool.tile([P, R], F16, name=f"x1_{m}") for m in range(DC)]
                bc_t = fpool.tile([P, R], F32, name="bc_t")
                xv_a = [fpool.tile([P, 2 * C2, D], F8, name=f"xv{gr}")
                        for gr in range(NT2 // 4)]

                with (
                    tc.tile_pool(name="xpool", bufs=1) as xpool,
                    tc.tile_pool(name="skpool", bufs=1) as skpool,
                ):
                    wqk_h = [xpool.tile([P, 4, D], F8, name=f"wqk_h{h}")
                             for h in range(2)]
                    xc8_h = [xpool.tile([P, 4, R], F8, name=f"xc8_h{h}")
                             for h in range(2)]
                    qw_h = [xpool.tile([P, QOFF + 4 * D], F16, name=f"qw_h{h}")
                            for h in range(2)]
                    xk_t = [skpool.tile([P, 2 * C2, 4 * P], F8, name=f"xk{g}")
                            for g in range(NT // 4)]

                    def ld_wqk(eng, h):
                        eng.dma_start(
                            wqk_h[h],
                            wqk_p[h * P:(h + 1) * P, :]
                            .rearrange("p (c e) -> p c e", c=4))

                    def ld_xc8(eng, h):
                        eng.dma_start(
                            xc8_h[h],
                            xc8_p[h * P:(h + 1) * P, :]
                            .rearrange("p (c r) -> p c r", c=4))

                    def ld_xk(eng, g):
                        eng.dma_start(
                            xk_t[g],
                            xk_p[g * P:(g + 1) * P, :]
                            .rearrange("p (c k) -> p c k", c=2 * C2))

                    def ld_xv(eng, gr):
                        eng.dma_start(
                            xv_a[gr],
                            xv_p[gr * P:(gr + 1) * P, :]
                            .rearrange("p (t e) -> p t e", t=2 * C2))

                    def ld_qw(eng, h):
                        eng.dma_start(qw_h[h], qw_p[h * P:(h + 1) * P, :])

                    # --- head dispatches: only sync/scalar/gpsimd can issue
                    # DMAs; per-queue deadline order ---
                    nc.vector.memset(ones8, 1.0)
                    nc.vector.memset(warm8, 1.0)
                    nc.vector.memset(ones_row, 1.0)

                    ld_wqk(nc.sync, 0)
                    ld_wqk(nc.sync, 1)
                    ld_xk(nc.sync, 0)
                    ld_xk(nc.sync, 3)
                    ld_xk(nc.sync, 7)
                    ld_qw(nc.sync, 0)
                    ld_xv(nc.sync, 0)
                    ld_xv(nc.sync, 3)

                    ld_xc8(nc.scalar, 0)
                    ld_xk(nc.scalar, 1)
                    ld_xk(nc.scalar, 5)
                    ld_qw(nc.scalar, 1)
                    ld_xv(nc.scalar, 1)

                    nc.gpsimd.dma_start(bias_t, bias_p[:, :])
                    ld_xc8(nc.gpsimd, 1)
                    ld_xk(nc.gpsimd, 2)
                    ld_xk(nc.gpsimd, 4)
                    ld_xk(nc.gpsimd, 6)
                    ld_xv(nc.gpsimd, 2)

                    # --- PE warm-up: dummy matmuls on memset tiles keep the
                    # HAM activity monitor busy so real work runs at 2.4GHz ---
                    warm_ps = pspool.tile([P, R], F32, name="warm_ps", tag="ps")
                    for _ in range(NDUMMY):
                        nc.tensor.matmul(
                            warm_ps, ones8, warm8, start=True, stop=True,
                            perf_mode=DR, skip_group_check=True)
                    nc.vector.tensor_copy(junk, warm_ps[0:1, 0:1])

                    # ============ G.T = (X_c @ Wqk + bqk).T (fp8 DR) ============
                    # c-outer for chunks 0/1 (start as data lands), then the
                    # last two chunks m-ordered so each g8 bias-add overlaps
                    # the remaining matmul tail.
                    g_ps = [pspool.tile([P, R], F32, name=f"gps{m}", tag="ps")
                            for m in range(DC)]

                    def g_stat(c, m):
                        lo = 2 * (c % 2)
                        return wqk_h[c // 2][:, lo:lo + 2, m * P:(m + 1) * P]

                    def g_mov(c):
                        lo = 2 * (c % 2)
                        return xc8_h[c // 2][:, lo:lo + 2, :]

                    for c in (0, 1):
                        for m in range(DC):
                            nc.tensor.matmul(
                                g_ps[m], g_stat(c, m), g_mov(c),
                                start=(c == 0), stop=False, perf_mode=DR)
                    for m in range(DC):
                        for c in (2, 3):
                            nc.tensor.matmul(
                                g_ps[m], g_stat(c, m), g_mov(c),
                                start=False, stop=(c == 3), perf_mode=DR)
                        if m % 2 == 0:
                            nc.vector.tensor_scalar_add(
                                g8[m // 2][:, m % 2, :], g_ps[m],
                                bias_t[:, m:m + 1])
                        else:
                            nc.scalar.add(
                                g8[m // 2][:, m % 2, :], g_ps[m],
                                bias_t[:, m:m + 1])

                    # ============ scores.T -> exp (fp8 DR) + sums ============
                    sums_ps = pspool.tile([P, R], F32, name="sums_ps", tag="ps")

                    def sums_mm(i):
                        nc.tensor.matmul(
                            sums_ps, ones8, e8[i],
                            start=(i == 0), stop=(i == NT2 - 1),
                            perf_mode=DR, skip_group_check=True)

                    for t in range(NT):
                        g, u = divmod(t, 4)
                        sc_ps = pspool.tile([P, R], F32, name="sc_ps", tag="ps")
                        for c in range(C2):
                            nc.tensor.matmul(
                                sc_ps,
                                xk_t[g][:, 2 * c:2 * c + 2, u * P:(u + 1) * P],
                                g8[c],
                                start=(c == 0), stop=(c == C2 - 1), perf_mode=DR)
                        nc.scalar.activation(
                            e8[t // 2][:, t % 2, :], sc_ps,
                            mybir.ActivationFunctionType.Exp,
                            bias=0.0, scale=1.0 / 32.0)
                        if t % 2 == 1 and t >= 3:
                            sums_mm((t - 3) // 2)   # one behind: that pair is done
                    sums_mm(NT2 - 1)
                    nc.vector.reciprocal(recip_t, sums_ps[0:1, :])

                    # ============ Q = (X_c @ Wq.T + bq).T (fp16) ============
                    # Runs while the scalar engine drains the exp tail and the
                    # DVE computes the reciprocal. k-outer; the last two
                    # k-steps go m-ordered so the bias-adds overlap.
                    q_ps = [pspool.tile([P, R], F32, name=f"qps{m}", tag="ps")
                            for m in range(DC)]

                    def q_stat(k, m):
                        h, kk = divmod(k, DC // 2)
                        lo = QOFF + kk * D + m * P
                        return qw_h[h][:, lo:lo + P]

                    def q_mov(k):
                        h, kk = divmod(k, DC // 2)
                        return qw_h[h][:, kk * R:(kk + 1) * R]

                    for m in range(DC):
                        for k in range(DC):
                            nc.tensor.matmul(
                                q_ps[m], q_stat(k, m), q_mov(k),
                                start=(k == 0), stop=(k == DC - 1))
                        if m % 2 == 0:
                            nc.vector.tensor_scalar_add(
                                qt_t[m], q_ps[m], bias_t[:, 8 + m:9 + m])
                        else:
                            nc.scalar.add(qt_t[m], q_ps[m], bias_t[:, 8 + m:9 + m])
                        if m == 0:
                            # broadcast 1/sums to all partitions here: recip is
                            # ready (no PE wait) and the bank+copy are long done
                            # before the U loop's PSUM allocations
                            bc_ps = pspool.tile([P, R], F32, name="bc_ps",
                                                tag="ps")
                            nc.tensor.matmul(bc_ps, ones_row, recip_t,
                                             start=True, stop=True)
                            nc.vector.tensor_copy(bc_t, bc_ps)

                with (
                    tc.tile_pool(name="lwpool", bufs=4) as lwpool,
                    tc.tile_pool(name="fwpool", bufs=3) as fwpool,
                ):
                    # late-phase weights (pre-tiled; DMAs start as soon as the
                    # early pools' SBUF frees up)
                    wv_a = lwpool.tile([P, 2 * C2, D], F8, name="wv_a", tag="lw")
                    wlv_a = lwpool.tile([P, 2 * C2, D], F8, name="wlv_a", tag="lw")
                    wav_a = lwpool.tile([P, 2 * C2, D], F8, name="wav_a", tag="lw")
                    wag_a = lwpool.tile([P, 2 * C2, D], F8, name="wag_a", tag="lw")
                    wl_a = fwpool.tile([P, DC, D], F16, name="wl_a", tag="fw")
                    wa_a = fwpool.tile([P, DC, D], F16, name="wa_a", tag="fw")
                    wg_a = fwpool.tile([P, DC, D], F16, name="wg_a", tag="fw")

                    def ld_w8(eng, tile_, b):
                        eng.dma_start(
                            tile_,
                            w8_p[b * P:(b + 1) * P, :]
                            .rearrange("p (c e) -> p c e", c=2 * C2))

                    def ld_w16(eng, tile_, b):
                        eng.dma_start(
                            tile_,
                            w16_p[b * P:(b + 1) * P, :]
                            .rearrange("p (k e) -> p k e", k=DC))

                    ld_w8(nc.sync, wv_a, 0)
                    ld_w16(nc.sync, wl_a, 0)
                    ld_w8(nc.scalar, wlv_a, 1)
                    ld_w8(nc.scalar, wag_a, 3)
                    ld_w8(nc.gpsimd, wav_a, 2)
                    ld_w16(nc.gpsimd, wa_a, 1)
                    ld_w16(nc.gpsimd, wg_a, 2)

                    # ============ U.T = (exp @ X).T (fp8 DR), normalize ========
                    vt_ps = [pspool.tile([P, R], F32, name=f"vtps{m}", tag="ps")
                             for m in range(DC)]
                    for t in range(NT2 - 1):
                        gr, u = divmod(t, 4)
                        for m in range(DC):
                            nc.tensor.matmul(
                                vt_ps[m],
                                xv_a[gr][:, 2 * u:2 * u + 2, m * P:(m + 1) * P],
                                e8[t],
                                start=(t == 0), stop=False,
                                perf_mode=DR, skip_group_check=True)
                    for m in range(DC):
                        nc.tensor.matmul(
                            vt_ps[m],
                            xv_a[3][:, 6:8, m * P:(m + 1) * P],
                            e8[NT2 - 1],
                            start=False, stop=True,
                            perf_mode=DR, skip_group_check=True)
                        nc.vector.tensor_mul(u8[m // 2][:, m % 2, :], vt_ps[m], bc_t)

                    # ============ V_.T = (U @ Wv.T + bv).T (fp8 DR) ============
                    v_ps = [pspool.tile([P, R], F32, name=f"vps{m}", tag="ps")
                            for m in range(DC)]
                    for c in (0, 1):
                        for m in range(DC):
                            nc.tensor.matmul(
                                v_ps[m], wv_a[:, 2 * c:2 * c + 2, m * P:(m + 1) * P],
                                u8[c],
                                start=(c == 0), stop=False, perf_mode=DR)
                    for m in range(DC):
                        for c in (2, 3):
                            nc.tensor.matmul(
                                v_ps[m], wv_a[:, 2 * c:2 * c + 2, m * P:(m + 1) * P],
                                u8[c],
                                start=False, stop=(c == 3), perf_mode=DR)
                        if m % 2 == 0:
                            nc.vector.tensor_scalar_add(
                                v8[m // 2][:, m % 2, :], v_ps[m],
                                bias_t[:, 16 + m:17 + m])
                        else:
                            nc.scalar.add(
                                v8[m // 2][:, m % 2, :], v_ps[m],
                                bias_t[:, 16 + m:17 + m])

                    # ========= x1 = [V_, Q] @ Wl.T + bl  (DR + fp16) =========
                    x1_ps = [pspool.tile([P, R], F32, name=f"x1ps{m}", tag="ps")
                             for m in range(DC)]
                    for c in range(C2):
                        for m in range(DC):
                            nc.tensor.matmul(
                                x1_ps[m],
                                wlv_a[:, 2 * c:2 * c + 2, m * P:(m + 1) * P], v8[c],
                                start=(c == 0), stop=False, perf_mode=DR)
                    for k in range(DC - 2):
                        for m in range(DC):
                            nc.tensor.matmul(
                                x1_ps[m], wl_a[:, k, m * P:(m + 1) * P], qt_t[k],
                                start=False, stop=False)
                    for m in range(DC):
                        for k in (DC - 2, DC - 1):
                            nc.tensor.matmul(
                                x1_ps[m], wl_a[:, k, m * P:(m + 1) * P], qt_t[k],
                                start=False, stop=(k == DC - 1))
                        if m % 2 == 0:
                            nc.vector.tensor_scalar_add(
                                x1_t[m], x1_ps[m], bias_t[:, 24 + m:25 + m])
                        else:
                            nc.scalar.add(x1_t[m], x1_ps[m], bias_t[:, 24 + m:25 + m])

                    # ========= h a-part; fold x1 <- x1 * (h_a + ba) =========
                    h_ps = [pspool.tile([P, R], F32, name=f"hps{m}", tag="ps")
                            for m in range(DC)]
                    for c in range(C2):
                        for m in range(DC):
                            nc.tensor.matmul(
                                h_ps[m],
                                wav_a[:, 2 * c:2 * c + 2, m * P:(m + 1) * P], v8[c],
                                start=(c == 0), stop=False, perf_mode=DR)
                    for k in range(DC - 2):
                        for m in range(DC):
                            nc.tensor.matmul(
                                h_ps[m], wa_a[:, k, m * P:(m + 1) * P], qt_t[k],
                                start=False, stop=False)
                    for m in range(DC):
                        for k in (DC - 2, DC - 1):
                            nc.tensor.matmul(
                                h_ps[m], wa_a[:, k, m * P:(m + 1) * P], qt_t[k],
                                start=False, stop=(k == DC - 1))
                        nc.vector.scalar_tensor_tensor(
                            x1_t[m], h_ps[m], bias_t[:, 32 + m:33 + m], x1_t[m],
                            op0=AD, op1=MU)

                    # ===== h b-part m-outer, sigmoid from PSUM, GLU, out =====
                    out_eng = [nc.sync, nc.gpsimd, nc.scalar, nc.gpsimd,
                               nc.scalar, nc.gpsimd, nc.scalar, None]
                    # last chunk halves: sync, then scalar (same engine as the
                    # sigmoid, so the dispatch follows it with no cross-engine
                    # hop)
                    hg1s = [pspool.tile([P, R], F32, name=f"hg1_{m}", tag="ps")
                            for m in range(DC)]
                    with tc.tile_pool(name="gpool", bufs=4) as gpool:
                        for m in range(DC):
                            hg1 = hg1s[m]
                            for c in range(C2):
                                nc.tensor.matmul(
                                    hg1,
                                    wag_a[:, 2 * c:2 * c + 2, m * P:(m + 1) * P],
                                    v8[c],
                                    start=(c == 0), stop=False, perf_mode=DR)
                            for k in range(DC):
                                nc.tensor.matmul(
                                    hg1, wg_a[:, k, m * P:(m + 1) * P], qt_t[k],
                                    start=False, stop=(k == DC - 1))
                            if m < DC - 1:
                                sig = gpool.tile([P, R], F16, name="sig", tag="g")
                                nc.scalar.activation(
                                    sig, hg1,
                                    mybir.ActivationFunctionType.Sigmoid,
                                    bias=bias_t[:, 40 + m:41 + m], scale=1.0)
                                og = gpool.tile([P, R], F16, name="og", tag="g")
                                nc.vector.tensor_mul(og, x1_t[m], sig)
                                out_eng[m].dma_start(
                                    out[m * P:(m + 1) * P, :], og)
                            else:
                                # last chunk: asymmetric 384/128 split so the
                                # final exposed piece is minimal
                                for hh, (lo, hi) in enumerate([(0, 384),
                                                               (384, R)]):
                                    sig = gpool.tile([P, hi - lo], F16,
                                                     name="sigh", tag="g")
                                    nc.scalar.activation(
                                        sig, hg1[:, lo:hi],
                                        mybir.ActivationFunctionType.Sigmoid,
                                        bias=bias_t[:, 40 + m:41 + m], scale=1.0)
                                    og = gpool.tile([P, hi - lo], F16,
                                                    name="ogh", tag="g")
                                    nc.vector.tensor_mul(
                                        og, x1_t[m][:, lo:hi], sig)
                                    eng = nc.sync if hh == 0 else nc.scalar
                                    eng.dma_start(
                                        out[m * P:(m + 1) * P, lo:hi], og)

    nc.compile()
    return nc


_NC = None


def _get_nc():
    global _NC
    if _NC is None:
        _NC = build_nc()
    return _NC


def make_in_maps(input_features, Wq, bq, Wk, bk, Wv, bv, Wl, bl, Wa, ba):
    f = np.ascontiguousarray
    x = np.asarray(input_features, dtype=np.float32)            # [N, D]
    x8 = x.astype(E4NP)                                          # [N, D]
    xt8 = x8.T                                                   # [D, N] view

    wq = np.asarray(Wq, np.float32)
    wk = np.asarray(Wk, np.float32)
    wqk8 = (wq.T @ wk).astype(E4NP)                              # [D, D]
    wqk_p = f(wqk8.reshape(2, 2, 2, P, D).transpose(0, 3, 1, 2, 4)
              .reshape(2 * P, 4 * D))
    xk_p = f(np.asarray(xt8).reshape(2 * C2, P, 8, 4 * P)
             .transpose(2, 1, 0, 3).reshape(8 * P, 8 * 4 * P))
    xv_p = f(x8.reshape(4, 2 * C2, P, D).transpose(0, 2, 1, 3)
             .reshape(4 * P, 8 * D))

    wvt = np.asarray(Wv, np.float32).T                           # [D, D]
    wlt = np.asarray(Wl, np.float32).T                           # [2D, D]
    wat = np.asarray(Wa, np.float32).T                           # [2D, 2D]

    def t8(w):
        return w.astype(E4NP).reshape(DC, P, D).transpose(1, 0, 2).reshape(P, DC * D)

    def t16(w):
        return (w.astype(np.float16).reshape(DC, P, D)
                .transpose(1, 0, 2).reshape(P, DC * D))

    w8_p = f(np.concatenate(
        [t8(wvt), t8(wlt[:D]), t8(wat[:D, :D]), t8(wat[:D, D:])], axis=0))
    w16_p = f(np.concatenate(
        [t16(wlt[D:]), t16(wat[D:, :D]), t16(wat[D:, D:])], axis=0))

    bqk = np.asarray(bq, np.float32) @ wk
    bias_arr = np.zeros((P, 48), np.float32)
    bias_arr[:, 0:8] = bqk.reshape(DC, P).T
    bias_arr[:, 8:16] = np.asarray(bq, np.float32).reshape(DC, P).T
    bias_arr[:, 16:24] = np.asarray(bv, np.float32).reshape(DC, P).T
    bias_arr[:, 24:32] = np.asarray(bl, np.float32).reshape(DC, P).T
    bias_arr[:, 32:48] = np.asarray(ba, np.float32).reshape(2 * DC, P).T
    bias_arr = f(bias_arr)

    wq16t = wq.T.astype(np.float16)                              # [D, D]
    wq_part = (wq16t.reshape(2, DC // 2, P, D).transpose(0, 2, 1, 3)
               .reshape(2 * P, (DC // 2) * D))                   # [256, 4D]
    xt16 = x.T.astype(np.float16)                                # [D, N]

    in_maps = []
    for c in range(NCORES):
        xc8 = np.asarray(xt8[:, c * R:(c + 1) * R])              # [D, R]
        xc8_pc = f(xc8.reshape(2, 2, 2, P, R).transpose(0, 3, 1, 2, 4)
                   .reshape(2 * P, 4 * R))
        xtc16 = xt16[:, c * R:(c + 1) * R]                       # [D, R]
        xt_part = (xtc16.reshape(2, DC // 2, P, R).transpose(0, 2, 1, 3)
                   .reshape(2 * P, (DC // 2) * R))
        qw_pc = f(np.concatenate([xt_part, wq_part], axis=1))    # [256, 4R+4D]
        in_maps.append({
            "wqk_p": wqk_p, "xc8_p": xc8_pc, "xk_p": xk_p, "xv_p": xv_p,
            "qw_p": qw_pc, "w8_p": w8_p, "w16_p": w16_p, "bias_p": bias_arr,
        })
    return in_maps


def run(in_maps, trace=False):
    nc = _get_nc()
    return bass_utils.run_bass_kernel_spmd(
        nc, in_maps, core_ids=list(range(NCORES)), trace=trace)


def kernel(input_features, Wq, bq, Wk, bk, Wv, bv, Wl, bl, Wa, ba):
    in_maps = make_in_maps(input_features, Wq, bq, Wk, bk, Wv, bv, Wl, bl, Wa, ba)
    res = run(in_maps)
    out = np.empty((N, D), dtype=np.float32)
    for c in range(NCORES):
        out[c * R:(c + 1) * R, :] = res.results[c]["out"].T.astype(np.float32)
    return out


# revision 21
# speedup vs baseline: 1.0417x; 1.0332x over previous
"""IntraAttention Trainium2 kernel, 8-core SPMD, collective-free.

Reference computation (N=4096 rows, d=1024):
    Q = X @ Wq.T + bq ; K = X @ Wk.T + bk ; V = X @ Wv.T + bv
    alpha = softmax(Q @ K.T / sqrt(d), axis=1)
    V_ = alpha @ V
    x = concat([V_, Q], axis=1)              # [N, 2d]
    x1 = x @ Wl.T + bl                        # [N, d]
    h = x @ Wa.T + ba                         # [N, 2d]
    out = x1 * (h[:, :d] * sigmoid(h[:, d:]))

Key algebraic restructuring (removes all collectives): every core holds
the FULL X (fp8) plus its row shard X_c, and uses
    scores = Q K.T = G @ X.T + softmax-invariant row-consts,
      with G = X_c @ Wqk + bqk, Wqk = Wq.T Wk (host-precomputed), and
    alpha V = (alpha @ X) @ Wv.T + bv      (rows of alpha sum to 1)
so K and V are never materialized or gathered.

Precision: attention path (G, scores, exp, U = exp@X, V_ = U@Wv.T, and
the V_-half of the x1/h projections) in fp8-e4m3 with DoubleRow
matmuls; the Q path stays fp16 (fp8 there blows the 2e-2 budget —
measured 5.4e-2 in emulation). End-to-end rel err ~3e-3.

Performance structure (170us vs the 189us baseline, whose trace showed
the first matmul at t=21.3us, two ~3us mid stalls, and an 8us tail):
  * All DRAM inputs are HOST-PRE-TILED into the exact SBUF tile layout,
    so every DMA descriptor is a 2-16KB contiguous per-partition run
    (the old rearranges generated 0.5-1KB strided descriptors; queue
    rate is ~45GB/s at 1-2KB rows vs 135-166GB/s at 4KB+).
  * All bulk loads dispatch up-front on the three DMA-capable engine
    queues (sync / scalar / gpsimd) in per-queue deadline order, G
    inputs first and combined into three 512KB transfers; the tiny
    bias load sits at the gpsimd queue head where it gates nothing.
  * 12 dummy matmuls on memset tiles warm the PE HAM clock gate to
    2.4GHz before the first real matmul (otherwise everything until
    t~24us runs at 1.2GHz); the PE then stays warm to the last matmul.
  * Accumulation loops finish per-output-column (last two K-steps
    m-ordered) so the PSUM->SBUF bias-adds overlap the matmul tail;
    the Q projection is m-outer so its PSUM banks don't WAR against
    the scalar engine's exp backlog from the scores loop; h-b PSUM
    banks are preallocated (per-iteration allocation cost ~400ns/m).
  * GLU tail: x1*a is folded into a fused (h+ba)*x1 DVE op during the
    h-a phase, sigmoid emits fp16, and the last output chunk is split
    into two half-R pieces on two queues (sync + scalar, scalar being
    the engine that just produced the sigmoid), shrinking the
    post-matmul serial chain from ~5us to ~2us.
"""

import numpy as np
import ml_dtypes

import concourse.bass as bass
import concourse.bacc as bacc
import concourse.tile as tile
import concourse.bass_utils as bass_utils
from concourse import mybir
from concourse.alu_op_type import AluOpType

P = 128            # partitions
D = 1024           # model dim
N = 4096           # rows
NCORES = 8
R = N // NCORES    # rows per core = 512
DC = D // P        # 128-wide d chunks = 8
C2 = D // (2 * P)  # 256-wide d chunks = 4
NT = N // P        # 128-key tiles = 32
NT2 = N // (2 * P)  # 256-key tiles = 16
TD = 2 * D
QOFF = 4 * R       # offset of the Wq half inside the flat qw tiles

F32 = mybir.dt.float32
F16 = mybir.dt.float16
F8 = mybir.dt.float8e4
DR = mybir.MatmulPerfMode.DoubleRow
E4NP = ml_dtypes.float8_e4m3fn
NDUMMY = 12

AD = AluOpType.add
MU = AluOpType.mult


def build_nc():
    nc = bacc.Bacc(
        "TRN2",
        target_bir_lowering=False,
        debug=False,
        num_devices=NCORES,
    )

    # ---- per-core I/O (all host-pre-tiled; see make_in_maps) ----
    wqk_p = nc.dram_tensor("wqk_p", [2 * P, 4 * D], F8, kind="ExternalInput")
    xc8_p = nc.dram_tensor("xc8_p", [2 * P, 4 * R], F8, kind="ExternalInput")
    xk_p = nc.dram_tensor("xk_p", [8 * P, 8 * 4 * P], F8, kind="ExternalInput")
    xv_p = nc.dram_tensor("xv_p", [4 * P, 8 * D], F8, kind="ExternalInput")
    qw_p = nc.dram_tensor("qw_p", [2 * P, QOFF + 4 * D], F16, kind="ExternalInput")
    w8_p = nc.dram_tensor("w8_p", [4 * P, 8 * D], F8, kind="ExternalInput")
    w16_p = nc.dram_tensor("w16_p", [3 * P, 8 * D], F16, kind="ExternalInput")
    bias_p = nc.dram_tensor("bias_p", [P, 48], F32, kind="ExternalInput")
    out = nc.dram_tensor("out", [D, R], F16, kind="ExternalOutput")

    with tile.TileContext(nc) as tc:
        with (
            tc.tile_pool(name="cpool", bufs=1) as cpool,
            tc.tile_pool(name="pspool", bufs=8, space="PSUM") as pspool,
        ):
            bias_t = cpool.tile([P, 48], F32, name="bias_t")
            ones8 = cpool.tile([P, 2, P], F8, name="ones8")
            ones_row = cpool.tile([1, P], F32, name="ones_row")
            warm8 = cpool.tile([P, 2, R], F8, name="warm8")
            junk = cpool.tile([1, 1], F32, name="junk")
            recip_t = cpool.tile([1, R], F32, name="recip_t")

            with (
                tc.tile_pool(name="qpool", bufs=1) as qpool,
                tc.tile_pool(name="e8pool", bufs=1) as e8pool,
                tc.tile_pool(name="fpool", bufs=1) as fpool,
            ):
                qt_t = [qpool.tile([P, R], F16, name=f"qt{m}") for m in range(DC)]
                g8 = [qpool.tile([P, 2, R], F8, name=f"g8_{c}") for c in range(C2)]
                e8 = [e8pool.tile([P, 2, R], F8, name=f"e8_{i}") for i in range(NT2)]
                u8 = [fpool.tile([P, 2, R], F8, name=f"u8_{c}") for c in range(C2)]
                v8 = [fpool.tile([P, 2, R], F8, name=f"v8_{c}") for c in range(C2)]
                x1_t = [fpool.tile([P, R], F16, name=f"x1_{m}") for m in range(DC)]
                bc_t = fpool.tile([P, R], F32, name="bc_t")
                xv_a = [fpool.tile([P, 2 * C2, D], F8, name=f"xv{gr}")
                        for gr in range(NT2 // 4)]

                with (
                    tc.tile_pool(name="xpool", bufs=1) as xpool,
                    tc.tile_pool(name="skpool", bufs=1) as skpool,
                ):
                    wqk_h = [xpool.tile([P, 4, D], F8, name=f"wqk_h{h}")
                             for h in range(2)]
                    xc8_h = [xpool.tile([P, 4, R], F8, name=f"xc8_h{h}")
                             for h in range(2)]
                    qw_h = [xpool.tile([P, QOFF + 4 * D], F16, name=f"qw_h{h}")
                            for h in range(2)]
                    xk_t = [skpool.tile([P, 2 * C2, 4 * P], F8, name=f"xk{g}")
                            for g in range(NT // 4)]

                    def ld_wqk(eng, h):
                        eng.dma_start(
                            wqk_h[h],
                            wqk_p[h * P:(h + 1) * P, :]
                            .rearrange("p (c e) -> p c e", c=4))

                    def ld_xc8(eng, h):
                        eng.dma_start(
                            xc8_h[h],
                            xc8_p[h * P:(h + 1) * P, :]
                            .rearrange("p (c r) -> p c r", c=4))

                    def ld_xk(eng, g):
                        eng.dma_start(
                            xk_t[g],
                            xk_p[g * P:(g + 1) * P, :]
                            .rearrange("p (c k) -> p c k", c=2 * C2))

                    def ld_xv(eng, gr):
                        eng.dma_start(
                            xv_a[gr],
                            xv_p[gr * P:(gr + 1) * P, :]
                            .rearrange("p (t e) -> p t e", t=2 * C2))

                    def ld_qw(eng, h):
                        eng.dma_start(qw_h[h], qw_p[h * P:(h + 1) * P, :])

                    # --- head dispatches: only sync/scalar/gpsimd can issue
                    # DMAs; per-queue deadline order ---
                    nc.vector.memset(ones8, 1.0)
                    nc.vector.memset(warm8, 1.0)
                    nc.vector.memset(ones_row, 1.0)

                    ld_wqk(nc.sync, 0)
                    ld_wqk(nc.sync, 1)
                    ld_xk(nc.sync, 0)
                    ld_xk(nc.sync, 3)
                    ld_xk(nc.sync, 7)
                    ld_qw(nc.sync, 0)
                    ld_xv(nc.sync, 0)
                    ld_xv(nc.sync, 3)

                    ld_xc8(nc.scalar, 0)
                    ld_xk(nc.scalar, 1)
                    ld_xk(nc.scalar, 5)
                    ld_qw(nc.scalar, 1)
                    ld_xv(nc.scalar, 1)

                    nc.gpsimd.dma_start(bias_t, bias_p[:, :])
                    ld_xc8(nc.gpsimd, 1)
                    ld_xk(nc.gpsimd, 2)
                    ld_xk(nc.gpsimd, 4)
                    ld_xk(nc.gpsimd, 6)
                    ld_xv(nc.gpsimd, 2)

                    # --- PE warm-up: dummy matmuls on memset tiles keep the
                    # HAM activity monitor busy so real work runs at 2.4GHz ---
                    warm_ps = pspool.tile([P, R], F32, name="warm_ps", tag="ps")
                    for _ in range(NDUMMY):
                        nc.tensor.matmul(
                            warm_ps, ones8, warm8, start=True, stop=True,
                            perf_mode=DR, skip_group_check=True)
                    nc.vector.tensor_copy(junk, warm_ps[0:1, 0:1])

                    # ============ G.T = (X_c @ Wqk + bqk).T (fp8 DR) ============
                    # c-outer for chunks 0/1 (start as data lands), then the
                    # last two chunks m-ordered so each g8 bias-add overlaps
                    # the remaining matmul tail.
                    g_ps = [pspool.tile([P, R], F32, name=f"gps{m}", tag="ps")
                            for m in range(DC)]

                    def g_stat(c, m):
                        lo = 2 * (c % 2)
                        return wqk_h[c // 2][:, lo:lo + 2, m * P:(m + 1) * P]

                    def g_mov(c):
                        lo = 2 * (c % 2)
                        return xc8_h[c // 2][:, lo:lo + 2, :]

                    for c in (0, 1):
                        for m in range(DC):
                            nc.tensor.matmul(
                                g_ps[m], g_stat(c, m), g_mov(c),
                                start=(c == 0), stop=False, perf_mode=DR)
                    for m in range(DC):
                        for c in (2, 3):
                            nc.tensor.matmul(
                                g_ps[m], g_stat(c, m), g_mov(c),
                                start=False, stop=(c == 3), perf_mode=DR)
                        if m % 2 == 0:
                            nc.vector.tensor_scalar_add(
                                g8[m // 2][:, m % 2, :], g_ps[m],
                                bias_t[:, m:m + 1])
                        else:
                            nc.scalar.add(
                                g8[m // 2][:, m % 2, :], g_ps[m],
                                bias_t[:, m:m + 1])

                    # ============ scores.T -> exp (fp8 DR) + sums ============
                    sums_ps = pspool.tile([P, R], F32, name="sums_ps", tag="ps")

                    def sums_mm(i):
                        nc.tensor.matmul(
                            sums_ps, ones8, e8[i],
                            start=(i == 0), stop=(i == NT2 - 1),
                            perf_mode=DR, skip_group_check=True)

                    for t in range(NT):
                        g, u = divmod(t, 4)
                        sc_ps = pspool.tile([P, R], F32, name="sc_ps", tag="ps")
                        for c in range(C2):
                            nc.tensor.matmul(
                                sc_ps,
                                xk_t[g][:, 2 * c:2 * c + 2, u * P:(u + 1) * P],
                                g8[c],
                                start=(c == 0), stop=(c == C2 - 1), perf_mode=DR)
                        nc.scalar.activation(
                            e8[t // 2][:, t % 2, :], sc_ps,
                            mybir.ActivationFunctionType.Exp,
                            bias=0.0, scale=1.0 / 32.0)
                        if t % 2 == 1 and t >= 3:
                            sums_mm((t - 3) // 2)   # one behind: that pair is done
                    sums_mm(NT2 - 1)
                    nc.vector.reciprocal(recip_t, sums_ps[0:1, :])

                    # ============ Q = (X_c @ Wq.T + bq).T (fp16) ============
                    # Runs while the scalar engine drains the exp tail and the
                    # DVE computes the reciprocal. k-outer; the last two
                    # k-steps go m-ordered so the bias-adds overlap.
                    q_ps = [pspool.tile([P, R], F32, name=f"qps{m}", tag="ps")
                            for m in range(DC)]

                    def q_stat(k, m):
                        h, kk = divmod(k, DC // 2)
                        lo = QOFF + kk * D + m * P
                        return qw_h[h][:, lo:lo + P]

                    def q_mov(k):
                        h, kk = divmod(k, DC // 2)
                        return qw_h[h][:, kk * R:(kk + 1) * R]

                    for m in range(DC):
                        for k in range(DC):
                            nc.tensor.matmul(
                                q_ps[m], q_stat(k, m), q_mov(k),
                                start=(k == 0), stop=(k == DC - 1))
                        if m % 2 == 0:
                            nc.vector.tensor_scalar_add(
                                qt_t[m], q_ps[m], bias_t[:, 8 + m:9 + m])
                        else:
                            nc.scalar.add(qt_t[m], q_ps[m], bias_t[:, 8 + m:9 + m])

                    # broadcast 1/sums to all partitions
                    bc_ps = pspool.tile([P, R], F32, name="bc_ps", tag="ps")
                    nc.tensor.matmul(bc_ps, ones_row, recip_t, start=True, stop=True)
                    nc.vector.tensor_copy(bc_t, bc_ps)

                with (
                    tc.tile_pool(name="lwpool", bufs=4) as lwpool,
                    tc.tile_pool(name="fwpool", bufs=3) as fwpool,
                ):
                    # late-phase weights (pre-tiled; DMAs start as soon as the
                    # early pools' SBUF frees up)
                    wv_a = lwpool.tile([P, 2 * C2, D], F8, name="wv_a", tag="lw")
                    wlv_a = lwpool.tile([P, 2 * C2, D], F8, name="wlv_a", tag="lw")
                    wav_a = lwpool.tile([P, 2 * C2, D], F8, name="wav_a", tag="lw")
                    wag_a = lwpool.tile([P, 2 * C2, D], F8, name="wag_a", tag="lw")
                    wl_a = fwpool.tile([P, DC, D], F16, name="wl_a", tag="fw")
                    wa_a = fwpool.tile([P, DC, D], F16, name="wa_a", tag="fw")
                    wg_a = fwpool.tile([P, DC, D], F16, name="wg_a", tag="fw")

                    def ld_w8(eng, tile_, b):
                        eng.dma_start(
                            tile_,
                            w8_p[b * P:(b + 1) * P, :]
                            .rearrange("p (c e) -> p c e", c=2 * C2))

                    def ld_w16(eng, tile_, b):
                        eng.dma_start(
                            tile_,
                            w16_p[b * P:(b + 1) * P, :]
                            .rearrange("p (k e) -> p k e", k=DC))

                    ld_w8(nc.sync, wv_a, 0)
                    ld_w16(nc.sync, wl_a, 0)
                    ld_w8(nc.scalar, wlv_a, 1)
                    ld_w8(nc.scalar, wag_a, 3)
                    ld_w8(nc.gpsimd, wav_a, 2)
                    ld_w16(nc.gpsimd, wa_a, 1)
                    ld_w16(nc.gpsimd, wg_a, 2)

                    # ============ U.T = (exp @ X).T (fp8 DR), normalize ========
                    vt_ps = [pspool.tile([P, R], F32, name=f"vtps{m}", tag="ps")
                             for m in range(DC)]
                    for t in range(NT2 - 1):
                        gr, u = divmod(t, 4)
                        for m in range(DC):
                            nc.tensor.matmul(
                                vt_ps[m],
                                xv_a[gr][:, 2 * u:2 * u + 2, m * P:(m + 1) * P],
                                e8[t],
                                start=(t == 0), stop=False,
                                perf_mode=DR, skip_group_check=True)
                    for m in range(DC):
                        nc.tensor.matmul(
                            vt_ps[m],
                            xv_a[3][:, 6:8, m * P:(m + 1) * P],
                            e8[NT2 - 1],
                            start=False, stop=True,
                            perf_mode=DR, skip_group_check=True)
                        nc.vector.tensor_mul(u8[m // 2][:, m % 2, :], vt_ps[m], bc_t)

                    # ============ V_.T = (U @ Wv.T + bv).T (fp8 DR) ============
                    v_ps = [pspool.tile([P, R], F32, name=f"vps{m}", tag="ps")
                            for m in range(DC)]
                    for c in (0, 1):
                        for m in range(DC):
                            nc.tensor.matmul(
                                v_ps[m], wv_a[:, 2 * c:2 * c + 2, m * P:(m + 1) * P],
                                u8[c],
                                start=(c == 0), stop=False, perf_mode=DR)
                    for m in range(DC):
                        for c in (2, 3):
                            nc.tensor.matmul(
                                v_ps[m], wv_a[:, 2 * c:2 * c + 2, m * P:(m + 1) * P],
                                u8[c],
                                start=False, stop=(c == 3), perf_mode=DR)
                        if m % 2 == 0:
                            nc.vector.tensor_scalar_add(
                                v8[m // 2][:, m % 2, :], v_ps[m],
                                bias_t[:, 16 + m:17 + m])
                        else:
                            nc.scalar.add(
                                v8[m // 2][:, m % 2, :], v_ps[m],
                                bias_t[:, 16 + m:17 + m])

                    # ========= x1 = [V_, Q] @ Wl.T + bl  (DR + fp16) =========
                    x1_ps = [pspool.tile([P, R], F32, name=f"x1ps{m}", tag="ps")
                             for m in range(DC)]
                    for c in range(C2):
                        for m in range(DC):
                            nc.tensor.matmul(
                                x1_ps[m],
                                wlv_a[:, 2 * c:2 * c + 2, m * P:(m + 1) * P], v8[c],
                                start=(c == 0), stop=False, perf_mode=DR)
                    for k in range(DC - 2):
                        for m in range(DC):
                            nc.tensor.matmul(
                                x1_ps[m], wl_a[:, k, m * P:(m + 1) * P], qt_t[k],
                                start=False, stop=False)
                    for m in range(DC):
                        for k in (DC - 2, DC - 1):
                            nc.tensor.matmul(
                                x1_ps[m], wl_a[:, k, m * P:(m + 1) * P], qt_t[k],
                                start=False, stop=(k == DC - 1))
                        if m % 2 == 0:
                            nc.vector.tensor_scalar_add(
                                x1_t[m], x1_ps[m], bias_t[:, 24 + m:25 + m])
                        else:
                            nc.scalar.add(x1_t[m], x1_ps[m], bias_t[:, 24 + m:25 + m])

                    # ========= h a-part; fold x1 <- x1 * (h_a + ba) =========
                    h_ps = [pspool.tile([P, R], F32, name=f"hps{m}", tag="ps")
                            for m in range(DC)]
                    for c in range(C2):
                        for m in range(DC):
                            nc.tensor.matmul(
                                h_ps[m],
                                wav_a[:, 2 * c:2 * c + 2, m * P:(m + 1) * P], v8[c],
                                start=(c == 0), stop=False, perf_mode=DR)
                    for k in range(DC - 2):
                        for m in range(DC):
                            nc.tensor.matmul(
                                h_ps[m], wa_a[:, k, m * P:(m + 1) * P], qt_t[k],
                                start=False, stop=False)
                    for m in range(DC):
                        for k in (DC - 2, DC - 1):
                            nc.tensor.matmul(
                                h_ps[m], wa_a[:, k, m * P:(m + 1) * P], qt_t[k],
                                start=False, stop=(k == DC - 1))
                        nc.vector.scalar_tensor_tensor(
                            x1_t[m], h_ps[m], bias_t[:, 32 + m:33 + m], x1_t[m],
                            op0=AD, op1=MU)

                    # ===== h b-part m-outer, sigmoid from PSUM, GLU, out =====
                    out_eng = [nc.sync, nc.gpsimd, nc.scalar, nc.gpsimd,
                               nc.scalar, nc.gpsimd, nc.scalar, None]
                    # last chunk halves: sync, then scalar (same engine as the
                    # sigmoid, so the dispatch follows it with no cross-engine
                    # hop)
                    hg1s = [pspool.tile([P, R], F32, name=f"hg1_{m}", tag="ps")
                            for m in range(DC)]
                    with tc.tile_pool(name="gpool", bufs=4) as gpool:
                        for m in range(DC):
                            hg1 = hg1s[m]
                            for c in range(C2):
                                nc.tensor.matmul(
                                    hg1,
                                    wag_a[:, 2 * c:2 * c + 2, m * P:(m + 1) * P],
                                    v8[c],
                                    start=(c == 0), stop=False, perf_mode=DR)
                            for k in range(DC):
                                nc.tensor.matmul(
                                    hg1, wg_a[:, k, m * P:(m + 1) * P], qt_t[k],
                                    start=False, stop=(k == DC - 1))
                            if m < DC - 1:
                                sig = gpool.tile([P, R], F16, name="sig", tag="g")
                                nc.scalar.activation(
                                    sig, hg1,
                                    mybir.ActivationFunctionType.Sigmoid,
                                    bias=bias_t[:, 40 + m:41 + m], scale=1.0)
                                og = gpool.tile([P, R], F16, name="og", tag="g")
                                nc.vector.tensor_mul(og, x1_t[m], sig)
                                out_eng[m].dma_start(
                                    out[m * P:(m + 1) * P, :], og)
                            else:
                                # last chunk: two half-R pieces on two queues
                                # to shrink the exposed post-matmul chain
                                for hh in range(2):
                                    lo, hi = hh * (R // 2), (hh + 1) * (R // 2)
                                    sig = gpool.tile([P, R // 2], F16,
                                                     name="sigh", tag="g")
                                    nc.scalar.activation(
                                        sig, hg1[:, lo:hi],
                                        mybir.ActivationFunctionType.Sigmoid,
                                        bias=bias_t[:, 40 + m:41 + m], scale=1.0)
                                    og = gpool.tile([P, R // 2], F16,
                                                    name="ogh", tag="g")
                                    nc.vector.tensor_mul(
                                        og, x1_t[m][:, lo:hi], sig)
                                    eng = nc.sync if hh == 0 else nc.scalar
                                    eng.dma_start(
                                        out[m * P:(m + 1) * P, lo:hi], og)

    nc.compile()
    return nc


_NC = None


def _get_nc():
    global _NC
    if _NC is None:
        _NC = build_nc()
    return _NC


def make_in_maps(input_features, Wq, bq, Wk, bk, Wv, bv, Wl, bl, Wa, ba):
    f = np.ascontiguousarray
    x = np.asarray(input_features, dtype=np.float32)            # [N, D]
    x8 = x.astype(E4NP)                                          # [N, D]
    xt8 = x8.T                                                   # [D, N] view

    wq = np.asarray(Wq, np.float32)
    wk = np.asarray(Wk, np.float32)
    wqk8 = (wq.T @ wk).astype(E4NP)                              # [D, D]
    wqk_p = f(wqk8.reshape(2, 2, 2, P, D).transpose(0, 3, 1, 2, 4)
              .reshape(2 * P, 4 * D))
    xk_p = f(np.asarray(xt8).reshape(2 * C2, P, 8, 4 * P)
             .transpose(2, 1, 0, 3).reshape(8 * P, 8 * 4 * P))
    xv_p = f(x8.reshape(4, 2 * C2, P, D).transpose(0, 2, 1, 3)
             .reshape(4 * P, 8 * D))

    wvt = np.asarray(Wv, np.float32).T                           # [D, D]
    wlt = np.asarray(Wl, np.float32).T                           # [2D, D]
    wat = np.asarray(Wa, np.float32).T                           # [2D, 2D]

    def t8(w):
        return w.astype(E4NP).reshape(DC, P, D).transpose(1, 0, 2).reshape(P, DC * D)

    def t16(w):
        return (w.astype(np.float16).reshape(DC, P, D)
                .transpose(1, 0, 2).reshape(P, DC * D))

    w8_p = f(np.concatenate(
        [t8(wvt), t8(wlt[:D]), t8(wat[:D, :D]), t8(wat[:D, D:])], axis=0))
    w16_p = f(np.concatenate(
        [t16(wlt[D:]), t16(wat[D:, :D]), t16(wat[D:, D:])], axis=0))

    bqk = np.asarray(bq, np.float32) @ wk
    bias_arr = np.zeros((P, 48), np.float32)
    bias_arr[:, 0:8] = bqk.reshape(DC, P).T
    bias_arr[:, 8:16] = np.asarray(bq, np.float32).reshape(DC, P).T
    bias_arr[:, 16:24] = np.asarray(bv, np.float32).reshape(DC, P).T
    bias_arr[:, 24:32] = np.asarray(bl, np.float32).reshape(DC, P).T
    bias_arr[:, 32:48] = np.asarray(ba, np.float32).reshape(2 * DC, P).T
    bias_arr = f(bias_arr)

    wq16t = wq.T.astype(np.float16)                              # [D, D]
    wq_part = (wq16t.reshape(2, DC // 2, P, D).transpose(0, 2, 1, 3)
               .reshape(2 * P, (DC // 2) * D))                   # [256, 4D]
    xt16 = x.T.astype(np.float16)                                # [D, N]

    in_maps = []
    for c in range(NCORES):
        xc8 = np.asarray(xt8[:, c * R:(c + 1) * R])              # [D, R]
        xc8_pc = f(xc8.reshape(2, 2, 2, P, R).transpose(0, 3, 1, 2, 4)
                   .reshape(2 * P, 4 * R))
        xtc16 = xt16[:, c * R:(c + 1) * R]                       # [D, R]
        xt_part = (xtc16.reshape(2, DC // 2, P, R).transpose(0, 2, 1, 3)
                   .reshape(2 * P, (DC // 2) * R))
        qw_pc = f(np.concatenate([xt_part, wq_part], axis=1))    # [256, 4R+4D]
        in_maps.append({
            "wqk_p": wqk_p, "xc8_p": xc8_pc, "xk_p": xk_p, "xv_p": xv_p,
            "qw_p": qw_pc, "w8_p": w8_p, "w16_p": w16_p, "bias_p": bias_arr,
        })
    return in_maps


def run(in_maps, trace=False):
    nc = _get_nc()
    return bass_utils.run_bass_kernel_spmd(
        nc, in_maps, core_ids=list(range(NCORES)), trace=trace)


def kernel(input_features, Wq, bq, Wk, bk, Wv, bv, Wl, bl, Wa, ba):
    in_maps = make_in_maps(input_features, Wq, bq, Wk, bk, Wv, bv, Wl, bl, Wa, ba)
    res = run(in_maps)
    out = np.empty((N, D), dtype=np.float32)
    for c in range(NCORES):
        out[c * R:(c + 1) * R, :] = res.results[c]["out"].T.astype(np.float32)
    return out
